# revision 21
# baseline (speedup 1.0000x reference)
"""Trainium2 Bass kernel for causal multi-head attention (B=4, T=2048, C=1024, H=16).

Sharding (8 cores, zero collectives): core c handles batch b=c//2 and head-half
half=c%2 (8 heads = 4 head pairs).  Each core:
  1. Q^T/K^T proj for its 8 heads (lhsT=W chunk, rhs=xT chunk), V proj in
     natural [k, d] layout; input DMAs split per 128-row chunk so the first
     projection matmuls start as soon as chunk 0 lands.
  2. causal flash attention in S^T orientation ([k partitions, q free]),
     two heads at a time: the even head's score matmul (K=64, PE rows 0-63)
     and the odd head's (rows 64-127) are emitted back-to-back so the PE's
     row-tiling runs them concurrently; one ScalarE exp per key chunk covers
     both heads ([128, 1024] PSUM tile); causal masking on diagonal 128x128
     blocks via a PE matmul-accumulate (S += eye.T @ mask); P^T bf16 -> per
     head O^T accumulation with a ones-column in V giving softmax row-sums in
     PSUM row 64; normalization: DVE reciprocal -> GpSimd partition_broadcast
     (Pool engine, keeps PE/DVE free) -> DVE multiply into y^T bf16; odd
     heads' y^T halves moved to partitions 64-127 by an SBUF->SBUF DMA.
  3. partial out^T = Wp_half^T @ y^T, emitted as fine-grained filler units
     (single matmuls) interleaved into the next q tile's attention stream,
     like the remaining QKV projection work, so the PE never idles while
     ScalarE runs exp.
Host: pre-transposes/casts x to x^T bf16 per batch, pre-scales Wq by D^-0.5,
slices weights per core; afterwards sums the two partial outputs per batch and
adds bp.  Biases bq/bk/bv (zeros in the spec) are supported via an augmented
ones-row contraction chunk, enabled only when they are nonzero.
"""

import os
import sys

import numpy as np

for _p in ("/opt/trn_rl_repo", "/root/.axon_site/_ro/trn_rl_repo"):
    if os.path.isdir(_p) and _p not in sys.path:
        sys.path.insert(0, _p)

import ml_dtypes  # noqa: E402

import concourse.bass as bass  # noqa: E402
import concourse.bacc as bacc  # noqa: E402
import concourse.mybir as mybir  # noqa: E402
import concourse.tile as tile  # noqa: E402

BF16 = mybir.dt.bfloat16
F32 = mybir.dt.float32
NEG = -1.0e30

C = 1024     # model dim
HALF = 512   # q/k/v columns per core (8 heads x 64)
HC = 8       # heads per core
D = 64       # head dim

_NC_CACHE: dict = {}


def _build_program(kc: int, T: int, reps: int = 1):
    """Single-core SPMD program.  kc = # of 128-row contraction chunks for the
    QKV projections (8, or 9 when biases are folded via an augmented row).
    reps>1 emits the body that many times (for steady-state HW timing)."""
    nc = bacc.Bacc("TRN2", target_bir_lowering=False)

    xT = nc.dram_tensor("xT", [kc * 128, T], BF16, kind="ExternalInput")
    wq = nc.dram_tensor("wq", [kc * 128, HALF], BF16, kind="ExternalInput")
    wk = nc.dram_tensor("wk", [kc * 128, HALF], BF16, kind="ExternalInput")
    wv = nc.dram_tensor("wv", [kc * 128, HALF], BF16, kind="ExternalInput")
    wp = nc.dram_tensor("wp", [HALF, C], BF16, kind="ExternalInput")
    mask = nc.dram_tensor("mask", [128, 128], BF16, kind="ExternalInput")
    eye = nc.dram_tensor("eye", [128, 128], BF16, kind="ExternalInput")
    outT = nc.dram_tensor("outT", [C, T], F32, kind="ExternalOutput")

    nqt = T // 512    # number of 512-wide query tiles
    nkr = T // 128    # number of 128-row key chunks

    with tile.TileContext(nc) as tc:
        with (
            tc.tile_pool(name="const", bufs=1) as const,
            tc.tile_pool(name="pt", bufs=3) as ptp,
            tc.tile_pool(name="rnorm", bufs=4) as rnp,
            tc.tile_pool(name="outb", bufs=3) as obp,
            tc.tile_pool(name="ps_s", bufs=2, space="PSUM") as pss,
            tc.tile_pool(name="ps_o", bufs=3, space="PSUM") as pso,
            tc.tile_pool(name="ps_w", bufs=1, space="PSUM") as psw,
        ):
            xt_sb = const.tile([128, kc, T], BF16, tag="xt")
            wq_sb = const.tile([128, kc, HALF], BF16, tag="wq")
            wk_sb = const.tile([128, kc, HALF], BF16, tag="wk")
            wv_sb = const.tile([128, kc, HALF], BF16, tag="wv")
            wp_sb = const.tile([128, 4, C], BF16, tag="wp")
            mask_sb = const.tile([128, 128], BF16, tag="mask")
            eye_sb = const.tile([128, 128], BF16, tag="eye")
            kt_sb = const.tile([128, 4, T], BF16, tag="kt")
            qt_sb = const.tile([128, 4, T], BF16, tag="qt")
            vx_sb = const.tile([128, nkr, HC, 65], BF16, tag="vx")
            yt_sb = [const.tile([128, T], BF16, tag=f"yt{i}", name=f"yt{i}")
                     for i in range(HC // 2)]

            ones_sb = const.tile([65, 64], BF16, tag="ones")

            nc.vector.memset(vx_sb[:, :, :, 64:65], 1.0)
            nc.vector.memset(ones_sb[64:65, :], 1.0)

            def emit_input_dmas():
                # x^T / Wq / Wk per contraction chunk so the initial Q/K
                # projection matmuls start as soon as chunk 0 lands
                for k in range(kc):
                    nc.sync.dma_start(out=xt_sb[:, k, :],
                                      in_=xT[128 * k:128 * k + 128, :])
                    nc.sync.dma_start(out=wq_sb[:, k, :],
                                      in_=wq[128 * k:128 * k + 128, :])
                    nc.sync.dma_start(out=wk_sb[:, k, :],
                                      in_=wk[128 * k:128 * k + 128, :])
                nc.sync.dma_start(
                    out=wv_sb[:],
                    in_=wv[:, :].rearrange("(k p) n -> p k n", p=128))
                nc.sync.dma_start(
                    out=wp_sb[:],
                    in_=wp[:, :].rearrange("(k p) n -> p k n", p=128))
                nc.sync.dma_start(out=mask_sb[:], in_=mask[:, :])
                nc.sync.dma_start(out=eye_sb[:], in_=eye[:, :])

            # ---- projection building blocks ----
            def proj_qk_group(w_sb, dst_sb, m, n, pool):
                ps = pool.tile([128, 512], F32, tag="ot")
                for k in range(kc):
                    nc.tensor.matmul(
                        ps[:, :],
                        w_sb[:, k, 128 * m:128 * m + 128],
                        xt_sb[:, k, 512 * n:512 * n + 512],
                        start=(k == 0), stop=(k == kc - 1))
                nc.vector.tensor_copy(
                    dst_sb[:, m, 512 * n:512 * n + 512], ps[:, :])

            def proj_v_group(kr, pool):
                ps = pool.tile([128, 512], F32, tag="ot")
                for k in range(kc):
                    nc.tensor.matmul(
                        ps[:, :],
                        xt_sb[:, k, 128 * kr:128 * kr + 128],
                        wv_sb[:, k, :],
                        start=(k == 0), stop=(k == kc - 1))
                nc.vector.tensor_copy(
                    vx_sb[:, kr, :, 0:64],
                    ps[:, :].rearrange("p (h e) -> p h e", e=64))

            # fine-grained filler units: each callable emits ONE instruction.
            # PSUM tiles are allocated at EMISSION time (inside the first
            # closure) so pool slot rotation matches program order.
            def queue_proj_qk(fillers, w_sb, dst_sb, m, n):
                cell = {}
                for k in range(kc):
                    def mm(k=k, cell=cell):
                        if k == 0:
                            cell["ps"] = psw.tile([128, 512], F32, tag="work",
                                                  name="fqk")
                        nc.tensor.matmul(
                            cell["ps"][:, :],
                            w_sb[:, k, 128 * m:128 * m + 128],
                            xt_sb[:, k, 512 * n:512 * n + 512],
                            start=(k == 0), stop=(k == kc - 1))
                    fillers.append(mm)
                fillers.append(lambda cell=cell: nc.vector.tensor_copy(
                    dst_sb[:, m, 512 * n:512 * n + 512], cell["ps"][:, :]))

            def queue_proj_v(fillers, kr):
                cell = {}
                for k in range(kc):
                    def mm(k=k, cell=cell):
                        if k == 0:
                            cell["ps"] = psw.tile([128, 512], F32, tag="work",
                                                  name="fv")
                        nc.tensor.matmul(
                            cell["ps"][:, :],
                            xt_sb[:, k, 128 * kr:128 * kr + 128],
                            wv_sb[:, k, :],
                            start=(k == 0), stop=(k == kc - 1))
                    fillers.append(mm)
                fillers.append(lambda cell=cell: nc.vector.tensor_copy(
                    vx_sb[:, kr, :, 0:64],
                    cell["ps"][:, :].rearrange("p (h e) -> p h e", e=64)))

            def queue_proj_out(fillers, qt, m):
                # runs through the 3-slot ot rotation so consecutive units'
                # matmuls overlap the previous unit's PSUM->SBUF copy
                cell = {}
                for k in range(4):
                    def mm(k=k, cell=cell):
                        if k == 0:
                            cell["ps"] = pso.tile([128, 512], F32, tag="ot",
                                                  name="fout")
                        nc.tensor.matmul(
                            cell["ps"][:, :],
                            wp_sb[:, k, 128 * m:128 * m + 128],
                            yt_sb[k][:, 512 * qt:512 * qt + 512],
                            start=(k == 0), stop=(k == 3))
                    fillers.append(mm)

                def finish(cell=cell):
                    ob = obp.tile([128, 512], F32, tag="ob")
                    nc.vector.tensor_copy(ob[:], cell["ps"][:, :])
                    nc.sync.dma_start(
                        out=outT[128 * m:128 * m + 128,
                                 512 * qt:512 * qt + 512],
                        in_=ob[:])
                fillers.append(finish)

            # ---- paired attention ----
            def attention_pair(hp, qt, emit_fillers):
                """Heads (2hp, 2hp+1) for query tile qt, score matmuls row-
                tiled so the two heads run concurrently on the PE."""
                nch = 4 * qt + 4
                he, ho = 2 * hp, 2 * hp + 1
                ot_e = pso.tile([128, 512], F32, tag="ot", name="ot_e")[0:65]
                ot_o = pso.tile([128, 512], F32, tag="ot", name="ot_o")[0:65]

                def emit_pv(j, qo, pt):
                    nc.tensor.matmul(
                        ot_e[:, qo:512],
                        vx_sb[:, j, he, :],
                        pt[:, qo:512],
                        start=(j == 0), stop=(j == nch - 1))
                    nc.tensor.matmul(
                        ot_o[:, qo:512],
                        vx_sb[:, j, ho, :],
                        pt[:, 512 + qo:1024],
                        start=(j == 0), stop=(j == nch - 1))

                pend = []   # deferred PV emissions: (j, qo, pt)
                for j in range(nch):
                    dj = j - 4 * qt
                    diag = dj >= 0
                    qo = 128 * dj if diag else 0
                    sm = pss.tile([128, 1024], F32, tag="sm")
                    # even head rows 0-63, odd head rows 64-127: row-tiled,
                    # concurrent on the PE
                    nc.tensor.matmul(
                        sm[:, qo:512],
                        kt_sb[0:64, hp, 128 * j:128 * j + 128],
                        qt_sb[0:64, hp, 512 * qt + qo:512 * qt + 512],
                        start=True, stop=not diag)
                    nc.tensor.matmul(
                        sm[:, 512 + qo:1024],
                        kt_sb[64:128, hp, 128 * j:128 * j + 128],
                        qt_sb[64:128, hp, 512 * qt + qo:512 * qt + 512],
                        start=True, stop=not diag)
                    if diag:
                        # causal mask on the boundary 128x128 block via a PE
                        # matmul-accumulate: S += eye.T @ mask = mask
                        nc.tensor.matmul(
                            sm[:, qo:qo + 128],
                            eye_sb[:, :], mask_sb[:, :],
                            start=False, stop=True)
                        nc.tensor.matmul(
                            sm[:, 512 + qo:512 + qo + 128],
                            eye_sb[:, :], mask_sb[:, :],
                            start=False, stop=True)
                    if pend:
                        emit_pv(*pend.pop(0))
                    pt = ptp.tile([128, 1024], BF16, tag="pt")
                    # exp both heads' valid column spans [qo:512], [512+qo:]
                    # in one instruction via a 3D access pattern (skips the
                    # uninitialized [512:512+qo) gap on diagonal chunks)
                    sm_v = sm.rearrange("p (g n) -> p g n", g=2)[:, :, qo:512]
                    pt_v = pt.rearrange("p (g n) -> p g n", g=2)[:, :, qo:512]
                    nc.scalar.activation(
                        out=pt_v, in_=sm_v,
                        func=mybir.ActivationFunctionType.Exp)
                    pend.append((j, qo, pt))
                    emit_fillers()

                while pend:
                    emit_pv(*pend.pop(0))

                # normalize: PSUM row 64 of ot_* holds the softmax denoms;
                # broadcast the reciprocal across partitions via a K=1 PE
                # matmul (gpsimd partition_broadcast is broken on this HW)
                for h, ot in ((he, ot_e), (ho, ot_o)):
                    rc = rnp.tile([65, 512], BF16, tag="rc")
                    with nc.allow_low_precision(
                            reason="softmax denom recip in bf16"):
                        nc.vector.reciprocal(rc[64:65, :], ot[64:65, :])
                    bc = psw.tile([64, 512], F32, tag="work", name="bc")
                    nc.tensor.matmul(bc[:, :], ones_sb[64:65, :],
                                     rc[64:65, :], start=True, stop=True)
                    rb = rnp.tile([64, 512], F32, tag="rb")
                    nc.vector.tensor_copy(rb[:], bc[:, :])
                    if h % 2 == 0:
                        nc.vector.tensor_mul(
                            yt_sb[h // 2][0:64, 512 * qt:512 * qt + 512],
                            ot[0:64, :], rb[:])
                    else:
                        yto = rnp.tile([64, 512], BF16, tag="yto")
                        nc.vector.tensor_mul(yto[:], ot[0:64, :], rb[:])
                        nc.sync.dma_start(
                            out=yt_sb[h // 2][64:128,
                                              512 * qt:512 * qt + 512],
                            in_=yto[:])

            # ---- one full execution ----
            def emit_body():
                emit_input_dmas()

                # initial projections, k-outer so each arriving x^T chunk
                # feeds 8 matmuls: Q's four accumulators live in the (not yet
                # used) score slots, K's in the ot slots + work slot
                smq = [pss.tile([128, 1024], F32, tag="sm", name=f"smq{i}")
                       for i in range(2)]
                q_ps = [smq[m // 2][:, 512 * (m % 2):512 * (m % 2) + 512]
                        for m in range(4)]
                k_ps = [pso.tile([128, 512], F32, tag="ot", name=f"kp{m}")
                        for m in range(3)]
                k_ps.append(psw.tile([128, 512], F32, tag="work", name="kp3"))
                for k in range(kc):
                    for m in range(4):
                        nc.tensor.matmul(
                            q_ps[m],
                            wq_sb[:, k, 128 * m:128 * m + 128],
                            xt_sb[:, k, 0:512],
                            start=(k == 0), stop=(k == kc - 1))
                    for m in range(4):
                        nc.tensor.matmul(
                            k_ps[m],
                            wk_sb[:, k, 128 * m:128 * m + 128],
                            xt_sb[:, k, 0:512],
                            start=(k == 0), stop=(k == kc - 1))
                for m in range(4):
                    nc.vector.tensor_copy(qt_sb[:, m, 0:512], q_ps[m])
                for m in range(4):
                    nc.vector.tensor_copy(kt_sb[:, m, 0:512], k_ps[m])
                for kr in range(min(4, nkr)):
                    proj_v_group(kr, pso)

                # filler queues, one per q tile, drained inside that tile's
                # attention.  queues[n-1] holds tile n's Q/K/V projections
                # (hard deadline: tile n's scores); all earlier tiles'
                # out-projections go to the LAST tile's queue, where the
                # ScalarE exp deficit is largest.
                queues = [[] for _ in range(nqt)]
                for n in range(1, nqt):
                    for m in range(4):
                        queue_proj_qk(queues[n - 1], wq_sb, qt_sb, m, n)
                    for m in range(4):
                        queue_proj_qk(queues[n - 1], wk_sb, kt_sb, m, n)
                    for kr in range(4 * n, min(4 * n + 4, nkr)):
                        queue_proj_v(queues[n - 1], kr)
                for qtp in range(nqt - 1):
                    for m in range(8):
                        queue_proj_out(queues[nqt - 1], qtp, m)

                for qt in range(nqt):
                    q = queues[qt]
                    iters = 4 * (4 * qt + 4)     # chunk-iters this q tile
                    done = [0]
                    total = len(q)

                    def make_emit(q=q, iters=iters, done=done, total=total):
                        it = [0]

                        def emit():
                            it[0] += 1
                            target = (total * it[0]) // iters
                            while done[0] < target and q:
                                q.pop(0)()
                                done[0] += 1
                        return emit

                    emit_fillers = make_emit()
                    for hp in range(4):
                        attention_pair(hp, qt, emit_fillers)
                    while q:
                        q.pop(0)()

                # final q tile's output projection (tail, through the ot
                # rotation so copies overlap the next unit's matmuls)
                fin = []
                for m in range(8):
                    queue_proj_out(fin, nqt - 1, m)
                for f in fin:
                    f()

            for _ in range(reps):
                emit_body()

    nc.finalize()
    return nc


def _prep_inputs(x, Wq, bq, Wk, bk, Wv, bv, Wp, bp, T):
    """Builds per-core in_maps.  Returns (in_maps, kc, use_bias)."""
    bf = ml_dtypes.bfloat16
    scale = 1.0 / np.sqrt(D)
    use_bias = bool(np.any(bq) or np.any(bk) or np.any(bv))
    kc = 9 if use_bias else 8

    mask_np = np.where(np.arange(128)[None, :] >= np.arange(128)[:, None],
                       np.float32(0.0), np.float32(NEG)).astype(bf)
    eye_np = np.eye(128, dtype=np.float32).astype(bf)

    def aug_x(xt):  # [1024, T] -> [kc*128, T]
        if not use_bias:
            return xt
        pad = np.zeros((128, xt.shape[1]), dtype=xt.dtype)
        pad[0, :] = 1.0
        return np.concatenate([xt, pad], axis=0)

    def aug_w(w, b):  # [1024, 512] -> [kc*128, 512]
        if not use_bias:
            return w
        pad = np.zeros((128, w.shape[1]), dtype=w.dtype)
        pad[0, :] = b
        return np.concatenate([w, pad], axis=0)

    in_maps = []
    for core in range(8):
        b = core // 2
        half = core % 2
        cs = slice(HALF * half, HALF * half + HALF)
        xt = np.ascontiguousarray(x[b, :T, :].T).astype(np.float32)
        in_maps.append({
            "xT": aug_x(xt).astype(bf),
            "wq": aug_w(Wq[:, cs] * scale, bq[cs] * scale).astype(bf),
            "wk": aug_w(Wk[:, cs], bk[cs]).astype(bf),
            "wv": aug_w(Wv[:, cs], bv[cs]).astype(bf),
            "wp": Wp[cs, :].astype(bf),
            "mask": mask_np,
            "eye": eye_np,
        })
    return in_maps, kc, use_bias


def run(inputs: dict, T: int = 2048, trace: bool = False, tmpdir=None):
    """Returns (output [B,T,C] f32, BassKernelResults)."""
    from concourse.bass_utils import run_bass_kernel_spmd

    x = np.asarray(inputs["x"], dtype=np.float32)
    B = x.shape[0]
    in_maps, kc, _ = _prep_inputs(
        x, *[np.asarray(inputs[k], dtype=np.float32) for k in
             ("Wq", "bq", "Wk", "bk", "Wv", "bv", "Wp", "bp")], T)

    key = (kc, T)
    if key not in _NC_CACHE:
        _NC_CACHE[key] = _build_program(kc, T)
    nc = _NC_CACHE[key]

    res = run_bass_kernel_spmd(nc, in_maps, list(range(8)),
                               trace=trace, tmpdir=tmpdir)

    bp = np.asarray(inputs["bp"], dtype=np.float32)
    out = np.empty((B, T, C), dtype=np.float32)
    for b in range(B):
        acc = res.results[2 * b]["outT"] + res.results[2 * b + 1]["outT"]
        out[b] = acc.T + bp[None, :]
    return out, res


def kernel(**inputs) -> np.ndarray:
    out, _ = run(inputs, T=2048, trace=False)
    return out


# revision 53
# speedup vs baseline: 1.5186x; 1.5186x over previous
"""Trainium2 Bass kernel for causal multi-head attention (B=4, T=2048, C=1024, H=16).

Sharding (8 cores, zero collectives): core c handles batch b=c//2 and head-half
half=c%2 (8 heads = 4 head pairs).  Each core:
  1. Q^T/K^T proj for its 8 heads (lhsT=W chunk, rhs=xT chunk), V proj in
     natural [k, d] layout; input DMAs split per 128-row chunk so the first
     projection matmuls start as soon as chunk 0 lands.
  2. causal flash attention in S^T orientation ([k partitions, q free]),
     two heads at a time: the even head's score matmul (K=64, PE rows 0-63)
     and the odd head's (rows 64-127) are emitted back-to-back so the PE's
     row-tiling runs them concurrently; one ScalarE exp per key chunk covers
     both heads ([128, 1024] PSUM tile); causal masking on diagonal 128x128
     blocks via a PE matmul-accumulate (S += eye.T @ mask); P^T bf16 -> per
     head O^T accumulation with a ones-column in V giving softmax row-sums in
     PSUM row 64; normalization: DVE reciprocal -> GpSimd partition_broadcast
     (Pool engine, keeps PE/DVE free) -> DVE multiply into y^T bf16; odd
     heads' y^T halves moved to partitions 64-127 by an SBUF->SBUF DMA.
  3. partial out^T = Wp_half^T @ y^T, emitted as fine-grained filler units
     (single matmuls) interleaved into the next q tile's attention stream,
     like the remaining QKV projection work, so the PE never idles while
     ScalarE runs exp.
Host: pre-transposes/casts x to x^T bf16 per batch, pre-scales Wq by D^-0.5,
slices weights per core; afterwards sums the two partial outputs per batch and
adds bp.  Biases bq/bk/bv (zeros in the spec) are supported via an augmented
ones-row contraction chunk, enabled only when they are nonzero.
"""

import os
import sys

import numpy as np

for _p in ("/opt/trn_rl_repo", "/root/.axon_site/_ro/trn_rl_repo"):
    if os.path.isdir(_p) and _p not in sys.path:
        sys.path.insert(0, _p)

import ml_dtypes  # noqa: E402

import concourse.bass as bass  # noqa: E402
import concourse.bacc as bacc  # noqa: E402
import concourse.mybir as mybir  # noqa: E402
import concourse.tile as tile  # noqa: E402

BF16 = mybir.dt.bfloat16
F32 = mybir.dt.float32
NEG = -1.0e30

C = 1024     # model dim
HALF = 512   # q/k/v columns per core (8 heads x 64)
HC = 8       # heads per core
D = 64       # head dim

_NC_CACHE: dict = {}


def _build_program(kc: int, T: int, reps: int = 1, serial: bool = False):
    """Single-core SPMD program.  kc = # of 128-row contraction chunks for the
    QKV projections (8, or 9 when biases are folded via an augmented row).
    reps>1 emits the body that many times; serial=True gates each body's
    input DMAs on the previous body's final output staging so chained bodies
    cannot overlap (approximates single-execution HW timing)."""
    nc = bacc.Bacc("TRN2", target_bir_lowering=False)

    xT = nc.dram_tensor("xT", [kc * 128, T], BF16, kind="ExternalInput")
    # Wq|Wk|Wv concatenated host-side: one 384KB DMA per contraction chunk
    wqkv = nc.dram_tensor("wqkv", [kc * 128, 3 * HALF], BF16,
                          kind="ExternalInput")
    wp = nc.dram_tensor("wp", [HALF, C], BF16, kind="ExternalInput")
    mask = nc.dram_tensor("mask", [128, 128], BF16, kind="ExternalInput")
    eye = nc.dram_tensor("eye", [128, 128], BF16, kind="ExternalInput")
    outT = nc.dram_tensor("outT", [C, T], F32, kind="ExternalOutput")

    nqt = T // 512    # number of 512-wide query tiles
    nkr = T // 128    # number of 128-row key chunks

    with tile.TileContext(nc) as tc:
        with (
            tc.tile_pool(name="const", bufs=1) as const,
            tc.tile_pool(name="pt", bufs=4) as ptp,
            tc.tile_pool(name="rnorm", bufs=4) as rnp,
            tc.tile_pool(name="outb", bufs=3) as obp,
            tc.tile_pool(name="ps_s", bufs=2, space="PSUM") as pss,
            tc.tile_pool(name="ps_o", bufs=2, space="PSUM") as pso,
            tc.tile_pool(name="ps_w", bufs=2, space="PSUM") as psw,
        ):
            xt_sb = const.tile([128, kc, T], BF16, tag="xt")
            wqkv_sb = const.tile([128, kc, 3 * HALF], BF16, tag="wqkv")
            wq_sb = wqkv_sb[:, :, 0:HALF]
            wk_sb = wqkv_sb[:, :, HALF:2 * HALF]
            wv_sb = wqkv_sb[:, :, 2 * HALF:3 * HALF]
            wp_sb = const.tile([128, 4, C], BF16, tag="wp")
            mask_sb = const.tile([128, 128], BF16, tag="mask")
            eye_sb = const.tile([128, 128], BF16, tag="eye")
            kt_sb = const.tile([128, 4, T], BF16, tag="kt")
            qt_sb = const.tile([128, 4, T], BF16, tag="qt")
            vx_sb = const.tile([128, nkr, HC, 65], BF16, tag="vx")
            yt_sb = [const.tile([128, T], BF16, tag=f"yt{i}", name=f"yt{i}")
                     for i in range(HC // 2)]

            ones_sb = const.tile([65, 64], BF16, tag="ones")

            nc.vector.memset(vx_sb[:, :, :, 64:65], 1.0)
            nc.vector.memset(ones_sb[64:65, :], 1.0)

            last_ob = {}

            def emit_serial_gate():
                # tiny DVE copies from the previous body's final output
                # staging tile into each input tile's corner: the input DMAs
                # then WAW-wait on them, serializing body boundaries
                src = last_ob["ob"][0:1, 0:1]
                nc.vector.tensor_copy(xt_sb[0:1, 0, 0:1], src)
                nc.vector.tensor_copy(wqkv_sb[0:1, 0, 0:1], src)
                nc.vector.tensor_copy(wp_sb[0:1, 0, 0:1], src)
                nc.vector.tensor_copy(mask_sb[0:1, 0:1], src)
                nc.vector.tensor_copy(eye_sb[0:1, 0:1], src)

            def emit_input_dmas():
                # Phase 1: exactly what the initial Q/K projections touch
                # (x^T cols 0-511 of each chunk + weight chunks), so the
                # first matmuls start ~1us in.  Phase 2: V weights, the x^T
                # column tails (in 512-col pieces, region deps let fillers
                # start on early pieces), then everything else.
                c1 = min(512, T)
                for k in range(kc):
                    nc.sync.dma_start(out=xt_sb[:, k, 0:c1],
                                      in_=xT[128 * k:128 * k + 128, 0:c1])
                    nc.sync.dma_start(out=wqkv_sb[:, k, :],
                                      in_=wqkv[128 * k:128 * k + 128, :])
                nc.sync.dma_start(out=mask_sb[:], in_=mask[:, :])
                nc.sync.dma_start(out=eye_sb[:], in_=eye[:, :])
                for k in range(kc):
                    if c1 < T:
                        nc.sync.dma_start(
                            out=xt_sb[:, k, c1:T],
                            in_=xT[128 * k:128 * k + 128, c1:T])
                nc.sync.dma_start(
                    out=wp_sb[:],
                    in_=wp[:, :].rearrange("(k p) n -> p k n", p=128))

            # ---- projection building blocks ----
            def proj_qk_group(w_sb, dst_sb, m, n, pool):
                ps = pool.tile([128, 512], F32,
                               tag="ot" if pool is pso else "work")
                for k in range(kc):
                    nc.tensor.matmul(
                        ps[:, :],
                        w_sb[:, k, 128 * m:128 * m + 128],
                        xt_sb[:, k, 512 * n:512 * n + 512],
                        start=(k == 0), stop=(k == kc - 1))
                nc.vector.tensor_copy(
                    dst_sb[:, m, 512 * n:512 * n + 512], ps[:, :])

            def proj_v_group(kr, pool):
                ps = pool.tile([128, 512], F32,
                               tag="ot" if pool is pso else "work")
                for k in range(kc):
                    nc.tensor.matmul(
                        ps[:, :],
                        xt_sb[:, k, 128 * kr:128 * kr + 128],
                        wv_sb[:, k, :],
                        start=(k == 0), stop=(k == kc - 1))
                nc.vector.tensor_copy(
                    vx_sb[:, kr, :, 0:64],
                    ps[:, :].rearrange("p (h e) -> p h e", e=64))

            # fine-grained filler units: each callable emits ONE instruction.
            # PSUM tiles are allocated at EMISSION time (inside the first
            # closure) so pool slot rotation matches program order.
            # filler units draw from the 2-slot work rotation so one unit's
            # PSUM->SBUF copy overlaps the next unit's matmuls
            def queue_proj_qk(fillers, w_sb, dst_sb, m, n):
                cell = {}
                for k in range(kc):
                    def mm(k=k, cell=cell):
                        if k == 0:
                            cell["ps"] = psw.tile([128, 512], F32,
                                                  tag="work", name="fqk")
                        nc.tensor.matmul(
                            cell["ps"][:, :],
                            w_sb[:, k, 128 * m:128 * m + 128],
                            xt_sb[:, k, 512 * n:512 * n + 512],
                            start=(k == 0), stop=(k == kc - 1))
                    fillers.append(mm)
                fillers.append(lambda cell=cell: nc.vector.tensor_copy(
                    dst_sb[:, m, 512 * n:512 * n + 512], cell["ps"][:, :]))

            def queue_proj_v(fillers, kr):
                cell = {}
                for k in range(kc):
                    def mm(k=k, cell=cell):
                        if k == 0:
                            cell["ps"] = psw.tile([128, 512], F32,
                                                  tag="work", name="fv")
                        nc.tensor.matmul(
                            cell["ps"][:, :],
                            xt_sb[:, k, 128 * kr:128 * kr + 128],
                            wv_sb[:, k, :],
                            start=(k == 0), stop=(k == kc - 1))
                    fillers.append(mm)
                fillers.append(lambda cell=cell: nc.vector.tensor_copy(
                    vx_sb[:, kr, :, 0:64],
                    cell["ps"][:, :].rearrange("p (h e) -> p h e", e=64)))

            def queue_proj_out(fillers, qt, m, copy_scalar=False):
                # runs through the 3-slot ot rotation so consecutive units'
                # matmuls overlap the previous unit's PSUM->SBUF copy
                cell = {}
                for k in range(4):
                    def mm(k=k, cell=cell):
                        if k == 0:
                            cell["ps"] = psw.tile([128, 512], F32,
                                                  tag="work", name="fout")
                        nc.tensor.matmul(
                            cell["ps"][:, :],
                            wp_sb[:, k, 128 * m:128 * m + 128],
                            yt_sb[k][:, 512 * qt:512 * qt + 512],
                            start=(k == 0), stop=(k == 3))
                    fillers.append(mm)

                def finish(cell=cell):
                    ob = obp.tile([128, 512], F32, tag="ob")
                    last_ob["ob"] = ob
                    if copy_scalar:
                        # tail copies alternate onto the (idle) ScalarE
                        nc.scalar.activation(
                            out=ob[:], in_=cell["ps"][:, :],
                            func=mybir.ActivationFunctionType.Copy)
                    else:
                        nc.vector.tensor_copy(ob[:], cell["ps"][:, :])
                    nc.sync.dma_start(
                        out=outT[128 * m:128 * m + 128,
                                 512 * qt:512 * qt + 512],
                        in_=ob[:])
                fillers.append(finish)

            # ---- paired attention ----
            def attention_pair(hp, qt, emit_fillers, carry_in):
                """Heads (2hp, 2hp+1) for query tile qt, score matmuls row-
                tiled so the two heads run concurrently on the PE.
                carry_in: previous pair's deferred normalize closures,
                emitted after this pair's first chunk so the PE bcast matmul
                hides under this pair's exp.  Returns this pair's normalize
                closures (carry_out)."""
                nch = 4 * qt + 4
                he, ho = 2 * hp, 2 * hp + 1
                ot_e = pso.tile([128, 512], F32, tag="ot", name="ot_e")[0:65]
                ot_o = pso.tile([128, 512], F32, tag="ot", name="ot_o")[0:65]

                def emit_pv(j, qo, pt):
                    nc.tensor.matmul(
                        ot_e[:, qo:512],
                        vx_sb[:, j, he, :],
                        pt[:, qo:512],
                        start=(j == 0), stop=(j == nch - 1))
                    nc.tensor.matmul(
                        ot_o[:, qo:512],
                        vx_sb[:, j, ho, :],
                        pt[:, 512 + qo:1024],
                        start=(j == 0), stop=(j == nch - 1))

                pend = []   # deferred PV emissions: (j, qo, pt)
                for j in range(nch):
                    dj = j - 4 * qt
                    diag = dj >= 0
                    qo = 128 * dj if diag else 0
                    sm = pss.tile([128, 1024], F32, tag="sm")
                    # even head rows 0-63, odd head rows 64-127: row-tiled,
                    # concurrent on the PE
                    nc.tensor.matmul(
                        sm[:, qo:512],
                        kt_sb[0:64, hp, 128 * j:128 * j + 128],
                        qt_sb[0:64, hp, 512 * qt + qo:512 * qt + 512],
                        start=True, stop=not diag)
                    nc.tensor.matmul(
                        sm[:, 512 + qo:1024],
                        kt_sb[64:128, hp, 128 * j:128 * j + 128],
                        qt_sb[64:128, hp, 512 * qt + qo:512 * qt + 512],
                        start=True, stop=not diag)
                    if diag:
                        # causal mask on the boundary 128x128 block via a PE
                        # matmul-accumulate: S += eye.T @ mask = mask
                        nc.tensor.matmul(
                            sm[:, qo:qo + 128],
                            eye_sb[:, :], mask_sb[:, :],
                            start=False, stop=True)
                        nc.tensor.matmul(
                            sm[:, 512 + qo:512 + qo + 128],
                            eye_sb[:, :], mask_sb[:, :],
                            start=False, stop=True)
                    if len(pend) >= 2:
                        # depth-2 deferral: PV(j-2) at chunk j gives exp(j-2)
                        # two chunks of PE work to complete
                        emit_pv(*pend.pop(0))
                    pt = ptp.tile([128, 1024], BF16, tag="pt")
                    # exp both heads' valid column spans [qo:512], [512+qo:]
                    # in one instruction via a 3D access pattern (skips the
                    # uninitialized [512:512+qo) gap on diagonal chunks)
                    sm_v = sm.rearrange("p (g n) -> p g n", g=2)[:, :, qo:512]
                    pt_v = pt.rearrange("p (g n) -> p g n", g=2)[:, :, qo:512]
                    nc.scalar.activation(
                        out=pt_v, in_=sm_v,
                        func=mybir.ActivationFunctionType.Exp)
                    pend.append((j, qo, pt))
                    if j == 0:
                        # previous pair's deferred normalize: after this
                        # pair's first scores+exp (hides the recip latency),
                        # before any filler that could claim its ot slot
                        for fn_ in carry_in:
                            fn_()
                        carry_in = []
                    emit_fillers()

                while pend:
                    emit_pv(*pend.pop(0))

                # normalize: PSUM row 64 of ot_* holds the softmax denoms;
                # broadcast the reciprocal across partitions via a K=1 PE
                # matmul (gpsimd partition_broadcast is broken on this HW).
                # Returned as closures so the caller can defer them into the
                # next pair's exp window.
                def normalize(h, ot):
                    rc = rnp.tile([65, 512], BF16, tag="rc", name="rc")
                    with nc.allow_low_precision(
                            reason="softmax denom recip in bf16"):
                        nc.vector.reciprocal(rc[64:65, :], ot[64:65, :])
                    bc = psw.tile([64, 512], F32, tag="work", name="bc")
                    nc.tensor.matmul(bc[:, :], ones_sb[64:65, :],
                                     rc[64:65, :], start=True, stop=True)
                    rb = rnp.tile([64, 512], F32, tag="rb", name="rb")
                    nc.vector.tensor_copy(rb[:], bc[:, :])
                    if h % 2 == 0:
                        nc.vector.tensor_mul(
                            yt_sb[h // 2][0:64, 512 * qt:512 * qt + 512],
                            ot[0:64, :], rb[:])
                    else:
                        yto = rnp.tile([64, 512], BF16, tag="yto", name="yto")
                        nc.vector.tensor_mul(yto[:], ot[0:64, :], rb[:])
                        nc.sync.dma_start(
                            out=yt_sb[h // 2][64:128,
                                              512 * qt:512 * qt + 512],
                            in_=yto[:])
                return [lambda: normalize(he, ot_e),
                        lambda: normalize(ho, ot_o)]

            # ---- one full execution ----
            def emit_body(gate=False):
                if gate:
                    emit_serial_gate()
                emit_input_dmas()

                # initial projections, k-outer so each arriving x^T chunk
                # feeds 8 matmuls: Q's four accumulators live in the (not yet
                # used) score slots, K's in the ot slots + work slot
                smq = [pss.tile([128, 1024], F32, tag="sm", name=f"smq{i}")
                       for i in range(2)]
                q_ps = [smq[m // 2][:, 512 * (m % 2):512 * (m % 2) + 512]
                        for m in range(4)]
                k_ps = [pso.tile([128, 512], F32, tag="ot", name=f"kp{m}")
                        for m in range(2)]
                k_ps += [psw.tile([128, 512], F32, tag="work", name=f"kp{m}")
                         for m in range(2, 4)]
                for k in range(kc):
                    for m in range(4):
                        nc.tensor.matmul(
                            q_ps[m],
                            wq_sb[:, k, 128 * m:128 * m + 128],
                            xt_sb[:, k, 0:512],
                            start=(k == 0), stop=(k == kc - 1))
                    for m in range(4):
                        nc.tensor.matmul(
                            k_ps[m],
                            wk_sb[:, k, 128 * m:128 * m + 128],
                            xt_sb[:, k, 0:512],
                            start=(k == 0), stop=(k == kc - 1))
                for m in range(4):
                    nc.vector.tensor_copy(qt_sb[:, m, 0:512], q_ps[m])
                for m in range(4):
                    nc.vector.tensor_copy(kt_sb[:, m, 0:512], k_ps[m])
                for kr in range(min(4, nkr)):
                    proj_v_group(kr, pso if kr % 2 == 0 else psw)

                # filler queues, one per q tile, drained inside that tile's
                # attention.  queues[n-1] holds tile n's Q/K/V projections
                # (hard deadline: tile n's scores); all earlier tiles'
                # out-projections go to the LAST tile's queue, where the
                # ScalarE exp deficit is largest.
                queues = [[] for _ in range(nqt)]
                for n in range(1, nqt):
                    for m in range(4):
                        queue_proj_qk(queues[n - 1], wq_sb, qt_sb, m, n)
                    for m in range(4):
                        queue_proj_qk(queues[n - 1], wk_sb, kt_sb, m, n)
                    for kr in range(4 * n, min(4 * n + 4, nkr)):
                        queue_proj_v(queues[n - 1], kr)
                for qtp in range(nqt - 1):
                    for m in range(8):
                        queue_proj_out(queues[nqt - 1], qtp, m)

                carry = []
                for qt in range(nqt):
                    q = queues[qt]
                    iters = 4 * (4 * qt + 4)     # chunk-iters this q tile
                    done = [0]
                    total = len(q)

                    def make_emit(q=q, iters=iters, done=done, total=total):
                        it = [0]

                        def emit():
                            it[0] += 1
                            target = (total * it[0]) // iters
                            while done[0] < target and q:
                                q.pop(0)()
                                done[0] += 1
                        return emit

                    emit_fillers = make_emit()
                    for hp in range(4):
                        carry = attention_pair(hp, qt, emit_fillers, carry)
                    if q:
                        # leftovers may need this qt's last ot slots: flush
                        # the deferred normalize first
                        for fn_ in carry:
                            fn_()
                        carry = []
                        while q:
                            q.pop(0)()
                for fn_ in carry:
                    fn_()

                # final q tile's output projection (tail, through the ot
                # rotation so copies overlap the next unit's matmuls; copies
                # alternate ScalarE/DVE since both are draining)
                fin = []
                for m in range(8):
                    queue_proj_out(fin, nqt - 1, m, copy_scalar=(m % 2 == 1))
                for f in fin:
                    f()

            for r in range(reps):
                emit_body(gate=serial and r > 0)

    nc.finalize()
    return nc


def _prep_inputs(x, Wq, bq, Wk, bk, Wv, bv, Wp, bp, T):
    """Builds per-core in_maps.  Returns (in_maps, kc, use_bias)."""
    bf = ml_dtypes.bfloat16
    scale = 1.0 / np.sqrt(D)
    use_bias = bool(np.any(bq) or np.any(bk) or np.any(bv))
    kc = 9 if use_bias else 8

    mask_np = np.where(np.arange(128)[None, :] >= np.arange(128)[:, None],
                       np.float32(0.0), np.float32(NEG)).astype(bf)
    eye_np = np.eye(128, dtype=np.float32).astype(bf)

    def aug_x(xt):  # [1024, T] -> [kc*128, T]
        if not use_bias:
            return xt
        pad = np.zeros((128, xt.shape[1]), dtype=xt.dtype)
        pad[0, :] = 1.0
        return np.concatenate([xt, pad], axis=0)

    def aug_w(w, b):  # [1024, 512] -> [kc*128, 512]
        if not use_bias:
            return w
        pad = np.zeros((128, w.shape[1]), dtype=w.dtype)
        pad[0, :] = b
        return np.concatenate([w, pad], axis=0)

    in_maps = []
    for core in range(8):
        b = core // 2
        half = core % 2
        cs = slice(HALF * half, HALF * half + HALF)
        xt = np.ascontiguousarray(x[b, :T, :].T).astype(np.float32)
        wqkv = np.concatenate([
            aug_w(Wq[:, cs] * scale, bq[cs] * scale),
            aug_w(Wk[:, cs], bk[cs]),
            aug_w(Wv[:, cs], bv[cs]),
        ], axis=1)
        in_maps.append({
            "xT": aug_x(xt).astype(bf),
            "wqkv": wqkv.astype(bf),
            "wp": Wp[cs, :].astype(bf),
            "mask": mask_np,
            "eye": eye_np,
        })
    return in_maps, kc, use_bias


def run(inputs: dict, T: int = 2048, trace: bool = False, tmpdir=None):
    """Returns (output [B,T,C] f32, BassKernelResults)."""
    from concourse.bass_utils import run_bass_kernel_spmd

    x = np.asarray(inputs["x"], dtype=np.float32)
    B = x.shape[0]
    in_maps, kc, _ = _prep_inputs(
        x, *[np.asarray(inputs[k], dtype=np.float32) for k in
             ("Wq", "bq", "Wk", "bk", "Wv", "bv", "Wp", "bp")], T)

    key = (kc, T)
    if key not in _NC_CACHE:
        _NC_CACHE[key] = _build_program(kc, T)
    nc = _NC_CACHE[key]

    res = run_bass_kernel_spmd(nc, in_maps, list(range(8)),
                               trace=trace, tmpdir=tmpdir)

    bp = np.asarray(inputs["bp"], dtype=np.float32)
    out = np.empty((B, T, C), dtype=np.float32)
    for b in range(B):
        acc = res.results[2 * b]["outT"] + res.results[2 * b + 1]["outT"]
        out[b] = acc.T + bp[None, :]
    return out, res


def kernel(**inputs) -> np.ndarray:
    out, _ = run(inputs, T=2048, trace=False)
    return out


# revision 54
# speedup vs baseline: 1.7671x; 1.1637x over previous
"""Trainium2 Bass kernel for causal multi-head attention (B=4, T=2048, C=1024, H=16).

Sharding (8 cores, zero collectives): core c handles batch b=c//2 and head-half
half=c%2 (8 heads = 4 head pairs).  Each core:
  1. Q^T/K^T proj for its 8 heads (lhsT=W chunk, rhs=xT chunk), V proj in
     natural [k, d] layout; input DMAs split per 128-row chunk so the first
     projection matmuls start as soon as chunk 0 lands.
  2. causal flash attention in S^T orientation ([k partitions, q free]),
     two heads at a time: the even head's score matmul (K=64, PE rows 0-63)
     and the odd head's (rows 64-127) are emitted back-to-back so the PE's
     row-tiling runs them concurrently; one ScalarE exp per key chunk covers
     both heads ([128, 1024] PSUM tile); causal masking on diagonal 128x128
     blocks via a PE matmul-accumulate (S += eye.T @ mask); P^T bf16 -> per
     head O^T accumulation with a ones-column in V giving softmax row-sums in
     PSUM row 64; normalization: DVE reciprocal -> GpSimd partition_broadcast
     (Pool engine, keeps PE/DVE free) -> DVE multiply into y^T bf16; odd
     heads' y^T halves moved to partitions 64-127 by an SBUF->SBUF DMA.
  3. partial out^T = Wp_half^T @ y^T, emitted as fine-grained filler units
     (single matmuls) interleaved into the next q tile's attention stream,
     like the remaining QKV projection work, so the PE never idles while
     ScalarE runs exp.
Host: pre-transposes/casts x to x^T bf16 per batch, pre-scales Wq by D^-0.5,
slices weights per core; afterwards sums the two partial outputs per batch and
adds bp.  Biases bq/bk/bv (zeros in the spec) are supported via an augmented
ones-row contraction chunk, enabled only when they are nonzero.
"""

import os
import sys

import numpy as np

for _p in ("/opt/trn_rl_repo", "/root/.axon_site/_ro/trn_rl_repo"):
    if os.path.isdir(_p) and _p not in sys.path:
        sys.path.insert(0, _p)

import ml_dtypes  # noqa: E402

import concourse.bass as bass  # noqa: E402
import concourse.bacc as bacc  # noqa: E402
import concourse.mybir as mybir  # noqa: E402
import concourse.tile as tile  # noqa: E402

BF16 = mybir.dt.bfloat16
F32 = mybir.dt.float32
NEG = -1.0e30

C = 1024     # model dim
HALF = 512   # q/k/v columns per core (8 heads x 64)
HC = 8       # heads per core
D = 64       # head dim

_NC_CACHE: dict = {}


def _build_program(kc: int, T: int, reps: int = 1, serial: bool = False):
    """Single-core SPMD program.  kc = # of 128-row contraction chunks for the
    QKV projections (8, or 9 when biases are folded via an augmented row).
    reps>1 emits the body that many times; serial=True gates each body's
    input DMAs on the previous body's final output staging so chained bodies
    cannot overlap (approximates single-execution HW timing)."""
    nc = bacc.Bacc("TRN2", target_bir_lowering=False)

    xT = nc.dram_tensor("xT", [kc * 128, T], BF16, kind="ExternalInput")
    # Wq|Wk|Wv concatenated host-side: one 384KB DMA per contraction chunk
    wqkv = nc.dram_tensor("wqkv", [kc * 128, 3 * HALF], BF16,
                          kind="ExternalInput")
    wp = nc.dram_tensor("wp", [HALF, C], BF16, kind="ExternalInput")
    mask = nc.dram_tensor("mask", [128, 128], BF16, kind="ExternalInput")
    eye = nc.dram_tensor("eye", [128, 128], BF16, kind="ExternalInput")
    outT = nc.dram_tensor("outT", [C, T], F32, kind="ExternalOutput")

    nqt = T // 512    # number of 512-wide query tiles
    nkr = T // 128    # number of 128-row key chunks

    with tile.TileContext(nc) as tc:
        with (
            tc.tile_pool(name="const", bufs=1) as const,
            tc.tile_pool(name="pt", bufs=4) as ptp,
            tc.tile_pool(name="rnorm", bufs=4) as rnp,
            tc.tile_pool(name="outb", bufs=3) as obp,
            tc.tile_pool(name="ps_s", bufs=2, space="PSUM") as pss,
            tc.tile_pool(name="ps_o", bufs=2, space="PSUM") as pso,
            tc.tile_pool(name="ps_w", bufs=2, space="PSUM") as psw,
        ):
            xt_sb = const.tile([128, kc, T], BF16, tag="xt")
            wqkv_sb = const.tile([128, kc, 3 * HALF], BF16, tag="wqkv")
            wq_sb = wqkv_sb[:, :, 0:HALF]
            wk_sb = wqkv_sb[:, :, HALF:2 * HALF]
            wv_sb = wqkv_sb[:, :, 2 * HALF:3 * HALF]
            wp_sb = const.tile([128, 4, C], BF16, tag="wp")
            mask_sb = const.tile([128, 128], BF16, tag="mask")
            eye_sb = const.tile([128, 128], BF16, tag="eye")
            kt_sb = const.tile([128, 4, T], BF16, tag="kt")
            qt_sb = const.tile([128, 4, T], BF16, tag="qt")
            vx_sb = const.tile([128, nkr, HC, 65], BF16, tag="vx")
            yt_sb = [const.tile([128, T], BF16, tag=f"yt{i}", name=f"yt{i}")
                     for i in range(HC // 2)]

            ones_sb = const.tile([65, 64], BF16, tag="ones")

            nc.vector.memset(vx_sb[:, :, :, 64:65], 1.0)
            nc.vector.memset(ones_sb[64:65, :], 1.0)

            last_ob = {}

            def emit_serial_gate():
                # tiny DVE copies from the previous body's final output
                # staging tile into each input tile's corner: the input DMAs
                # then WAW-wait on them, serializing body boundaries
                src = last_ob["ob"][0:1, 0:1]
                nc.vector.tensor_copy(xt_sb[0:1, 0, 0:1], src)
                nc.vector.tensor_copy(wqkv_sb[0:1, 0, 0:1], src)
                nc.vector.tensor_copy(wp_sb[0:1, 0, 0:1], src)
                nc.vector.tensor_copy(mask_sb[0:1, 0:1], src)
                nc.vector.tensor_copy(eye_sb[0:1, 0:1], src)

            def emit_input_dmas():
                # Phase 1: exactly what the initial Q/K projections touch
                # (x^T cols 0-511 of each chunk + weight chunks), so the
                # first matmuls start ~1us in.  Phase 2: V weights, the x^T
                # column tails (in 512-col pieces, region deps let fillers
                # start on early pieces), then everything else.
                c1 = min(512, T)
                for k in range(kc):
                    nc.sync.dma_start(out=xt_sb[:, k, 0:c1],
                                      in_=xT[128 * k:128 * k + 128, 0:c1])
                    nc.sync.dma_start(out=wqkv_sb[:, k, :],
                                      in_=wqkv[128 * k:128 * k + 128, :])
                nc.sync.dma_start(out=mask_sb[:], in_=mask[:, :])
                nc.sync.dma_start(out=eye_sb[:], in_=eye[:, :])
                if c1 < T:
                    c2 = min(c1 + 512, T)
                    for k in range(kc):
                        nc.sync.dma_start(
                            out=xt_sb[:, k, c1:c2],
                            in_=xT[128 * k:128 * k + 128, c1:c2])
                    for k in range(kc):
                        if c2 < T:
                            nc.sync.dma_start(
                                out=xt_sb[:, k, c2:T],
                                in_=xT[128 * k:128 * k + 128, c2:T])
                nc.sync.dma_start(
                    out=wp_sb[:],
                    in_=wp[:, :].rearrange("(k p) n -> p k n", p=128))

            # ---- projection building blocks ----
            def proj_qk_group(w_sb, dst_sb, m, n, pool):
                ps = pool.tile([128, 512], F32,
                               tag="ot" if pool is pso else "work")
                for k in range(kc):
                    nc.tensor.matmul(
                        ps[:, :],
                        w_sb[:, k, 128 * m:128 * m + 128],
                        xt_sb[:, k, 512 * n:512 * n + 512],
                        start=(k == 0), stop=(k == kc - 1))
                nc.vector.tensor_copy(
                    dst_sb[:, m, 512 * n:512 * n + 512], ps[:, :])

            def proj_v_group(kr, pool):
                ps = pool.tile([128, 512], F32,
                               tag="ot" if pool is pso else "work")
                for k in range(kc):
                    nc.tensor.matmul(
                        ps[:, :],
                        xt_sb[:, k, 128 * kr:128 * kr + 128],
                        wv_sb[:, k, :],
                        start=(k == 0), stop=(k == kc - 1))
                nc.vector.tensor_copy(
                    vx_sb[:, kr, :, 0:64],
                    ps[:, :].rearrange("p (h e) -> p h e", e=64))

            # fine-grained filler units: each callable emits ONE instruction.
            # PSUM tiles are allocated at EMISSION time (inside the first
            # closure) so pool slot rotation matches program order.
            # filler units draw from the 2-slot work rotation so one unit's
            # PSUM->SBUF copy overlaps the next unit's matmuls
            def queue_proj_qk(fillers, w_sb, dst_sb, m, n):
                cell = {}
                for k in range(kc):
                    def mm(k=k, cell=cell):
                        if k == 0:
                            cell["ps"] = psw.tile([128, 512], F32,
                                                  tag="work", name="fqk")
                        nc.tensor.matmul(
                            cell["ps"][:, :],
                            w_sb[:, k, 128 * m:128 * m + 128],
                            xt_sb[:, k, 512 * n:512 * n + 512],
                            start=(k == 0), stop=(k == kc - 1))
                    fillers.append(mm)
                fillers.append(lambda cell=cell: nc.vector.tensor_copy(
                    dst_sb[:, m, 512 * n:512 * n + 512], cell["ps"][:, :]))

            def queue_proj_v(fillers, kr):
                cell = {}
                for k in range(kc):
                    def mm(k=k, cell=cell):
                        if k == 0:
                            cell["ps"] = psw.tile([128, 512], F32,
                                                  tag="work", name="fv")
                        nc.tensor.matmul(
                            cell["ps"][:, :],
                            xt_sb[:, k, 128 * kr:128 * kr + 128],
                            wv_sb[:, k, :],
                            start=(k == 0), stop=(k == kc - 1))
                    fillers.append(mm)
                fillers.append(lambda cell=cell: nc.vector.tensor_copy(
                    vx_sb[:, kr, :, 0:64],
                    cell["ps"][:, :].rearrange("p (h e) -> p h e", e=64)))

            def queue_proj_out(fillers, qt, m, copy_scalar=False):
                # runs through the 3-slot ot rotation so consecutive units'
                # matmuls overlap the previous unit's PSUM->SBUF copy
                cell = {}
                for k in range(4):
                    def mm(k=k, cell=cell):
                        if k == 0:
                            cell["ps"] = psw.tile([128, 512], F32,
                                                  tag="work", name="fout")
                        nc.tensor.matmul(
                            cell["ps"][:, :],
                            wp_sb[:, k, 128 * m:128 * m + 128],
                            yt_sb[k][:, 512 * qt:512 * qt + 512],
                            start=(k == 0), stop=(k == 3))
                    fillers.append(mm)

                def finish(cell=cell):
                    ob = obp.tile([128, 512], F32, tag="ob")
                    last_ob["ob"] = ob
                    if copy_scalar:
                        # tail copies alternate onto the (idle) ScalarE
                        nc.scalar.activation(
                            out=ob[:], in_=cell["ps"][:, :],
                            func=mybir.ActivationFunctionType.Copy)
                    else:
                        nc.vector.tensor_copy(ob[:], cell["ps"][:, :])
                    nc.sync.dma_start(
                        out=outT[128 * m:128 * m + 128,
                                 512 * qt:512 * qt + 512],
                        in_=ob[:])
                fillers.append(finish)

            # ---- paired attention ----
            def attention_pair(hp, qt, emit_fillers, carry_in):
                """Heads (2hp, 2hp+1) for query tile qt, score matmuls row-
                tiled so the two heads run concurrently on the PE.
                carry_in: previous pair's deferred normalize closures,
                emitted after this pair's first chunk so the PE bcast matmul
                hides under this pair's exp.  Returns this pair's normalize
                closures (carry_out)."""
                nch = 4 * qt + 4
                he, ho = 2 * hp, 2 * hp + 1
                ot_e = pso.tile([128, 512], F32, tag="ot", name="ot_e")[0:65]
                ot_o = pso.tile([128, 512], F32, tag="ot", name="ot_o")[0:65]

                def emit_pv(j, qo, pt):
                    nc.tensor.matmul(
                        ot_e[:, qo:512],
                        vx_sb[:, j, he, :],
                        pt[:, qo:512],
                        start=(j == 0), stop=(j == nch - 1))
                    nc.tensor.matmul(
                        ot_o[:, qo:512],
                        vx_sb[:, j, ho, :],
                        pt[:, 512 + qo:1024],
                        start=(j == 0), stop=(j == nch - 1))

                pend = []   # deferred PV emissions: (j, qo, pt)
                for j in range(nch):
                    dj = j - 4 * qt
                    diag = dj >= 0
                    qo = 128 * dj if diag else 0
                    sm = pss.tile([128, 1024], F32, tag="sm")
                    # even head rows 0-63, odd head rows 64-127: row-tiled,
                    # concurrent on the PE
                    nc.tensor.matmul(
                        sm[:, qo:512],
                        kt_sb[0:64, hp, 128 * j:128 * j + 128],
                        qt_sb[0:64, hp, 512 * qt + qo:512 * qt + 512],
                        start=True, stop=not diag)
                    nc.tensor.matmul(
                        sm[:, 512 + qo:1024],
                        kt_sb[64:128, hp, 128 * j:128 * j + 128],
                        qt_sb[64:128, hp, 512 * qt + qo:512 * qt + 512],
                        start=True, stop=not diag)
                    if diag:
                        # causal mask on the boundary 128x128 block via a PE
                        # matmul-accumulate: S += eye.T @ mask = mask
                        nc.tensor.matmul(
                            sm[:, qo:qo + 128],
                            eye_sb[:, :], mask_sb[:, :],
                            start=False, stop=True)
                        nc.tensor.matmul(
                            sm[:, 512 + qo:512 + qo + 128],
                            eye_sb[:, :], mask_sb[:, :],
                            start=False, stop=True)
                    if len(pend) >= 2:
                        # depth-2 deferral: PV(j-2) at chunk j gives exp(j-2)
                        # two chunks of PE work to complete
                        emit_pv(*pend.pop(0))
                    pt = ptp.tile([128, 1024], BF16, tag="pt")
                    # exp both heads' valid column spans [qo:512], [512+qo:]
                    # in one instruction via a 3D access pattern (skips the
                    # uninitialized [512:512+qo) gap on diagonal chunks)
                    sm_v = sm.rearrange("p (g n) -> p g n", g=2)[:, :, qo:512]
                    pt_v = pt.rearrange("p (g n) -> p g n", g=2)[:, :, qo:512]
                    nc.scalar.activation(
                        out=pt_v, in_=sm_v,
                        func=mybir.ActivationFunctionType.Exp)
                    pend.append((j, qo, pt))
                    if j == 0:
                        # previous pair's deferred normalize: after this
                        # pair's first scores+exp (hides the recip latency),
                        # before any filler that could claim its ot slot
                        for fn_ in carry_in:
                            fn_()
                        carry_in = []
                    emit_fillers()

                while pend:
                    emit_pv(*pend.pop(0))

                # normalize: PSUM row 64 of ot_* holds the softmax denoms;
                # broadcast the reciprocal across partitions via a K=1 PE
                # matmul (gpsimd partition_broadcast is broken on this HW).
                # Returned as closures so the caller can defer them into the
                # next pair's exp window.
                def normalize(h, ot):
                    rc = rnp.tile([65, 512], BF16, tag="rc", name="rc")
                    with nc.allow_low_precision(
                            reason="softmax denom recip in bf16"):
                        nc.vector.reciprocal(rc[64:65, :], ot[64:65, :])
                    bc = psw.tile([64, 512], F32, tag="work", name="bc")
                    nc.tensor.matmul(bc[:, :], ones_sb[64:65, :],
                                     rc[64:65, :], start=True, stop=True)
                    rb = rnp.tile([64, 512], F32, tag="rb", name="rb")
                    nc.vector.tensor_copy(rb[:], bc[:, :])
                    if h % 2 == 0:
                        nc.vector.tensor_mul(
                            yt_sb[h // 2][0:64, 512 * qt:512 * qt + 512],
                            ot[0:64, :], rb[:])
                    else:
                        yto = rnp.tile([64, 512], BF16, tag="yto", name="yto")
                        nc.vector.tensor_mul(yto[:], ot[0:64, :], rb[:])
                        nc.sync.dma_start(
                            out=yt_sb[h // 2][64:128,
                                              512 * qt:512 * qt + 512],
                            in_=yto[:])
                return [lambda: normalize(he, ot_e),
                        lambda: normalize(ho, ot_o)]

            # ---- one full execution ----
            def emit_body(gate=False):
                if gate:
                    emit_serial_gate()
                emit_input_dmas()

                # initial projections, k-outer so each arriving x^T chunk
                # feeds 8 matmuls: Q's four accumulators live in the (not yet
                # used) score slots, K's in the ot slots + work slot
                smq = [pss.tile([128, 1024], F32, tag="sm", name=f"smq{i}")
                       for i in range(2)]
                q_ps = [smq[m // 2][:, 512 * (m % 2):512 * (m % 2) + 512]
                        for m in range(4)]
                k_ps = [pso.tile([128, 512], F32, tag="ot", name=f"kp{m}")
                        for m in range(2)]
                k_ps += [psw.tile([128, 512], F32, tag="work", name=f"kp{m}")
                         for m in range(2, 4)]
                for k in range(kc):
                    for m in range(4):
                        nc.tensor.matmul(
                            q_ps[m],
                            wq_sb[:, k, 128 * m:128 * m + 128],
                            xt_sb[:, k, 0:512],
                            start=(k == 0), stop=(k == kc - 1))
                    for m in range(4):
                        nc.tensor.matmul(
                            k_ps[m],
                            wk_sb[:, k, 128 * m:128 * m + 128],
                            xt_sb[:, k, 0:512],
                            start=(k == 0), stop=(k == kc - 1))
                for m in range(4):
                    nc.vector.tensor_copy(qt_sb[:, m, 0:512], q_ps[m])
                for m in range(4):
                    nc.vector.tensor_copy(kt_sb[:, m, 0:512], k_ps[m])
                for kr in range(min(4, nkr)):
                    proj_v_group(kr, pso if kr % 2 == 0 else psw)

                # filler queues, one per q tile, drained inside that tile's
                # attention.  queues[n-1] holds tile n's Q/K/V projections
                # (hard deadline: tile n's scores); all earlier tiles'
                # out-projections go to the LAST tile's queue, where the
                # ScalarE exp deficit is largest.
                queues = [[] for _ in range(nqt)]
                for n in range(1, nqt):
                    for m in range(4):
                        queue_proj_qk(queues[n - 1], wq_sb, qt_sb, m, n)
                    for m in range(4):
                        queue_proj_qk(queues[n - 1], wk_sb, kt_sb, m, n)
                    for kr in range(4 * n, min(4 * n + 4, nkr)):
                        queue_proj_v(queues[n - 1], kr)
                for qtp in range(nqt - 1):
                    for m in range(8):
                        queue_proj_out(queues[nqt - 1], qtp, m)

                carry = []
                for qt in range(nqt):
                    q = queues[qt]
                    iters = 4 * (4 * qt + 4)     # chunk-iters this q tile
                    done = [0]
                    total = len(q)

                    def make_emit(q=q, iters=iters, done=done, total=total):
                        it = [0]

                        def emit():
                            it[0] += 1
                            target = (total * it[0]) // iters
                            while done[0] < target and q:
                                q.pop(0)()
                                done[0] += 1
                        return emit

                    emit_fillers = make_emit()
                    for hp in range(4):
                        carry = attention_pair(hp, qt, emit_fillers, carry)
                    if q:
                        # leftovers may need this qt's last ot slots: flush
                        # the deferred normalize first
                        for fn_ in carry:
                            fn_()
                        carry = []
                        while q:
                            q.pop(0)()
                for fn_ in carry:
                    fn_()

                # final q tile's output projection (tail, through the ot
                # rotation so copies overlap the next unit's matmuls; copies
                # alternate ScalarE/DVE since both are draining)
                fin = []
                for m in range(8):
                    queue_proj_out(fin, nqt - 1, m, copy_scalar=(m % 2 == 1))
                for f in fin:
                    f()

            for r in range(reps):
                emit_body(gate=serial and r > 0)

    nc.finalize()
    return nc


def _prep_inputs(x, Wq, bq, Wk, bk, Wv, bv, Wp, bp, T):
    """Builds per-core in_maps.  Returns (in_maps, kc, use_bias)."""
    bf = ml_dtypes.bfloat16
    scale = 1.0 / np.sqrt(D)
    use_bias = bool(np.any(bq) or np.any(bk) or np.any(bv))
    kc = 9 if use_bias else 8

    mask_np = np.where(np.arange(128)[None, :] >= np.arange(128)[:, None],
                       np.float32(0.0), np.float32(NEG)).astype(bf)
    eye_np = np.eye(128, dtype=np.float32).astype(bf)

    def aug_x(xt):  # [1024, T] -> [kc*128, T]
        if not use_bias:
            return xt
        pad = np.zeros((128, xt.shape[1]), dtype=xt.dtype)
        pad[0, :] = 1.0
        return np.concatenate([xt, pad], axis=0)

    def aug_w(w, b):  # [1024, 512] -> [kc*128, 512]
        if not use_bias:
            return w
        pad = np.zeros((128, w.shape[1]), dtype=w.dtype)
        pad[0, :] = b
        return np.concatenate([w, pad], axis=0)

    in_maps = []
    for core in range(8):
        b = core // 2
        half = core % 2
        cs = slice(HALF * half, HALF * half + HALF)
        xt = np.ascontiguousarray(x[b, :T, :].T).astype(np.float32)
        wqkv = np.concatenate([
            aug_w(Wq[:, cs] * scale, bq[cs] * scale),
            aug_w(Wk[:, cs], bk[cs]),
            aug_w(Wv[:, cs], bv[cs]),
        ], axis=1)
        in_maps.append({
            "xT": aug_x(xt).astype(bf),
            "wqkv": wqkv.astype(bf),
            "wp": Wp[cs, :].astype(bf),
            "mask": mask_np,
            "eye": eye_np,
        })
    return in_maps, kc, use_bias


def run(inputs: dict, T: int = 2048, trace: bool = False, tmpdir=None):
    """Returns (output [B,T,C] f32, BassKernelResults)."""
    from concourse.bass_utils import run_bass_kernel_spmd

    x = np.asarray(inputs["x"], dtype=np.float32)
    B = x.shape[0]
    in_maps, kc, _ = _prep_inputs(
        x, *[np.asarray(inputs[k], dtype=np.float32) for k in
             ("Wq", "bq", "Wk", "bk", "Wv", "bv", "Wp", "bp")], T)

    key = (kc, T)
    if key not in _NC_CACHE:
        _NC_CACHE[key] = _build_program(kc, T)
    nc = _NC_CACHE[key]

    res = run_bass_kernel_spmd(nc, in_maps, list(range(8)),
                               trace=trace, tmpdir=tmpdir)

    bp = np.asarray(inputs["bp"], dtype=np.float32)
    out = np.empty((B, T, C), dtype=np.float32)
    for b in range(B):
        acc = res.results[2 * b]["outT"] + res.results[2 * b + 1]["outT"]
        out[b] = acc.T + bp[None, :]
    return out, res


def kernel(**inputs) -> np.ndarray:
    out, _ = run(inputs, T=2048, trace=False)
    return out
